# revision 42
# baseline (speedup 1.0000x reference)
"""Lorenz96 RK4 integrator on TRN2 — 8-core data parallel Bass kernel (v2).

Math: integrate dx_i/dt = (x_{i+1} - x_{i-2}) * x_{i-1} - x_i + F (cyclic,
F=8) from t=0 to t=1 for 262144 independent trajectories of dim 40.

v2 strategy (vs the v1 19-pass fp32 STT kernel):
- Integrating factor: s = e^t x turns the ODE into ds/dt = a(t)*N(s) + b(t)
  with N(s) = (roll(s,-1)-roll(s,2))*roll(s,1) (degree-2 homogeneous),
  a = e^-t, b = F e^t.  The "- x + F" part of the derivative disappears
  into per-stage compile-time scalars, so a classic RK4 step needs only
  15 tensor-tensor passes per element on the owning engine plus 7
  scalar-affine passes (w_i = c_i a_i m + c_i b_i, z_i likewise) that ride
  the Activation engine's free scale*x+bias path.
- fp16 on the DVE chunks: plain tensor_tensor supports the 2x_1p DVE perf
  mode for 2-byte dtypes (STT does not, which is why v1 could not use it).
  fp16 noise is ~1e-3 of the final error budget (measured: N=12 fp16 err
  1.03e-2 vs the 2e-2 gate; truncation dominates).
- dim-major layout [P, DIM, C] for the fp16 chunks: cyclic shifts become
  slices along the middle (dim) axis, so every operand keeps innermost
  stride 1 / count C and stays 4-byte aligned (C even) -> 2x mode holds
  for every shifted op on real HW, not just in the cost model.
- Pool (GpSimd) chunks stay fp32 in row-major [P, C, DIM] (Pool's Q7 cost
  is dtype-independent; fp32 avoids any Q7 fp16 risk), with ACT doing
  their w/z affine ops too.
- N_STEPS = 12 (error 1.03e-2 < 2e-2; N=11 at 1.7e-2 is too thin).
- Engine balance per step (per core): DVE 15 passes on 204/256 row-blocks
  at 0.52 ns/elem, Pool 15 passes on 52/256 at 1.98 ns/elem, ACT 7 passes
  on all 256 at 0.83 ns/elem -> all three ~62-67 us/step.
"""

import math

import numpy as np

F_FORCE = 8.0
T_END = 1.0
BATCH, DIM = 262144, 40
N_CORES = 8
ROWS = BATCH // N_CORES  # rows per core
P = 128                  # SBUF partitions
RB = ROWS // P           # row-blocks per partition (256)

N_STEPS = 10
RHO = 0.96  # geometric step-size ratio (dt_n ~ RHO^n, normalized to sum 1)
DT = T_END / N_STEPS

# rows-per-partition chunk sizes (sum must equal RB); keep C even so the
# fp16 dim-slices stay 4B-aligned.
DVE_CHUNKS = (102, 110)   # fp16 dim-major chunks owned by the Vector engine
GP_CHUNKS = (44,)      # fp32 row-major chunks owned by the Pool engine

_CACHE: dict = {}
LABELS: dict = {}  # instruction name -> human label (diagnostics)


def _lab(inst, label):
    try:
        LABELS[inst.ins.name] = label
    except Exception:
        pass
    return inst


class _ActChain:
    """Force the Tile scheduler to keep ACT instructions in emission order
    via ordering-only (no-sync) dependencies.  Tile schedules each engine's
    static order with its own internal cost model; when two independent
    compute paths share ACT, a pacing mismatch lets one path's affine ops
    pile up ahead of the other's in the static order, which then starves
    the other path at runtime (observed: paths drifting 4 steps apart and
    ~50us stalls).  Chaining pins the order so both paths stay in lockstep.
    """

    def __init__(self):
        self.last = None

    def __call__(self, inst):
        from concourse.instruction_name_ordered_set import (
            InstructionNameOrderedSet,
        )
        if self.last is not None:
            s = InstructionNameOrderedSet()
            s.add(self.last)
            inst.ins.add_nosync_dependencies_from(s)
        self.last = inst.ins.name
        return inst


def build(n_steps=N_STEPS, rows=ROWS, dve_chunks=DVE_CHUNKS,
          gp_chunks=GP_CHUNKS, rho=RHO, act_interleave=True,
          pool_w_self=True, pe_assist=True):
    """Build the Bass module for one core's shard ([rows, DIM] in -> out).

    w_on_dve: compute the DVE chunks' w-affine on DVE via tensor_scalar
      (4x fp16 mode) instead of ACT, removing ACT from the y critical path.
    act_interleave: order ACT's per-stage ops DVE/Pool interleaved instead
      of all-DVE-then-all-Pool.
    """
    import concourse.mybir as mybir
    from concourse import bacc, bass, tile
    from concourse.masks import make_identity

    f16 = mybir.dt.float16
    f32 = mybir.dt.float32
    Copy = mybir.ActivationFunctionType.Copy

    rb = rows // P
    assert sum(dve_chunks) + sum(gp_chunks) == rb
    assert all(C % 2 == 0 for C in dve_chunks)

    # Geometric step schedule: dt_n ~ rho^n (sum = T_END).  Late-step local
    # error dominates the final error for this system, so rho slightly
    # below 1 (late steps smaller) buys accuracy for free.
    wts = [rho ** k for k in range(n_steps)]
    dts = [T_END * w / sum(wts) for w in wts]

    # RK4 stage constants (classic): y2 = s + (dt/2)k1, y3 = s + (dt/2)k2,
    # y4 = s + dt*k3, s' = s + sum(g_i k_i); k_i = a_i*m_i + b_i in s-space.
    delta = (0.0, 0.5, 0.5, 1.0)

    nc = bacc.Bacc("TRN2", target_bir_lowering=False, debug=False)
    # Host-marshalled I/O: the host converts to fp16 and lays each chunk
    # out exactly as its SBUF tile (dim-major [P,DIM,C] for DVE chunks,
    # row-major [P,C,DIM] for Pool chunks), so the device runs no
    # conversion passes and DMA bytes are halved.
    xio = {}
    for j, C in enumerate(dve_chunks):
        xio[f"xd{j}"] = nc.dram_tensor(f"xd{j}", [P, DIM, C], f16,
                                       kind="ExternalInput")
        xio[f"yd{j}"] = nc.dram_tensor(f"yd{j}", [P, DIM, C], f16,
                                       kind="ExternalOutput")
    for j, C in enumerate(gp_chunks):
        xio[f"xg{j}"] = nc.dram_tensor(f"xg{j}", [P, C, DIM], f16,
                                       kind="ExternalInput")
        xio[f"yg{j}"] = nc.dram_tensor(f"yg{j}", [P, C, DIM], f16,
                                       kind="ExternalOutput")

    with tile.TileContext(nc) as tc:
        with tc.tile_pool(name="work", bufs=1) as pool, \
             tc.tile_pool(name="acc", space=bass.MemorySpace.PSUM,
                          bufs=1) as ppool:

            # ---------------- allocate chunks, issue input DMAs ----------
            off = 0
            gstates = []
            for j, C in enumerate(gp_chunks):
                s = {
                    "C": C, "off": off, "j": f"g{j}", "io": f"g{j}",
                    # s gets the DMA directly (fp16 row-major state)
                    "s": pool.tile([P, C, DIM], f16, tag=f"s_g{j}",
                                   name=f"s_g{j}"),
                    "y": pool.tile([P, C, DIM], f16, tag=f"y_g{j}",
                                   name=f"y_g{j}"),
                    "t1": pool.tile([P, C, DIM], f16, tag=f"t1_g{j}",
                                    name=f"t1_g{j}"),
                    "w": pool.tile([P, C, DIM], f16, tag=f"w_g{j}",
                                   name=f"w_g{j}"),
                    "A": pool.tile([P, C, DIM], f16, tag=f"A_g{j}",
                                   name=f"A_g{j}"),
                    "z": pool.tile([P, C, DIM], f16, tag=f"z_g{j}",
                                   name=f"z_g{j}"),
                }
                gstates.append(s)
                off += C
            # PE-assist machinery: the first DVE chunk's z-accumulation
            # A = sum_i (g_i a_i) m_i runs on the otherwise-idle TensorE as
            # scaled-identity matmuls accumulating into PSUM; ACT extracts
            # B = A + sum_i g_i b_i.  PSUM (16 KiB/partition = 4096 fp32)
            # fits one C=102 chunk (4080 fp32).
            ident = wtile = psumA = None
            if pe_assist:
                assert dve_chunks and dve_chunks[0] * DIM <= 4096
                ident = pool.tile([P, P], f16, tag="ident", name="ident")
                wtile = pool.tile([P, P], f16, tag="W", bufs=2, name="W")
                psumA = ppool.tile([P, dve_chunks[0] * DIM], f32, tag="A_pe",
                                   name="A_pe")
            dstates = []
            for j, C in enumerate(dve_chunks):
                pe = pe_assist and j == 0
                s = {
                    "C": C, "off": off, "j": j, "io": f"d{j}",
                    "s": pool.tile([P, DIM, C], f16, tag=f"s_d{j}",
                                   name=f"s_d{j}"),
                    "y": pool.tile([P, DIM, C], f16, tag=f"y_d{j}",
                                   name=f"y_d{j}"),
                    "t1": pool.tile([P, DIM, C], f16, tag=f"t1_d{j}",
                                    name=f"t1_d{j}"),
                    "w": pool.tile([P, DIM, C], f16, tag=f"w_d{j}",
                                   name=f"w_d{j}"),
                }
                if not pe:
                    # the PE chunk accumulates in PSUM: no A/z tiles
                    s["A"] = pool.tile([P, DIM, C], f16, tag=f"A_d{j}",
                                       name=f"A_d{j}")
                    s["z"] = pool.tile([P, DIM, C], f16, tag=f"z_d{j}",
                                       name=f"z_d{j}")
                dstates.append(s)
                off += C

            def fresh_m(st, dim_major):
                # rotate the m tile per stage (bufs=2): the next stage's
                # shift write never waits on ACT's z still reading the
                # previous m.  The PE chunk has no ACT z reader, so a
                # single buffer suffices there.
                j = st.get("j", st["off"])
                nb = 1 if (pe_assist and dstates and st is dstates[0]) else 2
                shape = [P, DIM, st["C"]] if dim_major else [P, st["C"], DIM]
                st["m"] = pool.tile(shape, f16, tag=f"m_{dim_major}_{j}",
                                    bufs=nb, name=f"m_{j}")
                return st["m"]

            # Input DMAs: straight into the state tiles (host already
            # converted and transposed).  Spread across both HWDGE queues.
            nc.sync.dma_start(dstates[0]["s"][:, :, :], xio["xd0"][:, :, :])
            for j, g in enumerate(gstates):
                nc.scalar.dma_start(g["s"][:, :, :], xio[f"xg{j}"][:, :, :])
            for j, d in enumerate(dstates[1:], start=1):
                nc.sync.dma_start(d["s"][:, :, :], xio[f"xd{j}"][:, :, :])

            if pe_assist:
                make_identity(nc, ident[:, :])

            # ---------------- shift helpers ------------------------------
            def shifts_d(st, v, tag=""):
                # dim-major fp16: slices along the middle (dim) axis.
                t1, m = st["t1"], fresh_m(st, True)
                eng = nc.vector
                # t1 = roll(v,-1) - roll(v,2)
                _lab(eng.tensor_sub(t1[:, 0:2, :], v[:, 1:3, :], v[:, 38:40, :]), f"t1a{tag}")
                _lab(eng.tensor_sub(t1[:, 2:39, :], v[:, 3:40, :], v[:, 0:37, :]), f"t1b{tag}")
                _lab(eng.tensor_sub(t1[:, 39:40, :], v[:, 0:1, :], v[:, 37:38, :]), f"t1c{tag}")
                # m = t1 * roll(v,1)
                _lab(eng.tensor_mul(m[:, 0:1, :], t1[:, 0:1, :], v[:, 39:40, :]), f"ma{tag}")
                _lab(eng.tensor_mul(m[:, 1:40, :], t1[:, 1:40, :], v[:, 0:39, :]), f"mb{tag}")

            def shifts_g(st, v, tag=""):
                # row-major fp32: slices along the last (dim) axis.
                t1, m = st["t1"], fresh_m(st, False)
                eng = nc.gpsimd
                _lab(eng.tensor_sub(t1[:, :, 0:2], v[:, :, 1:3], v[:, :, 38:40]), f"t1a{tag}")
                _lab(eng.tensor_sub(t1[:, :, 2:39], v[:, :, 3:40], v[:, :, 0:37]), f"t1b{tag}")
                _lab(eng.tensor_sub(t1[:, :, 39:40], v[:, :, 0:1], v[:, :, 37:38]), f"t1c{tag}")
                _lab(eng.tensor_mul(m[:, :, 0:1], t1[:, :, 0:1], v[:, :, 39:40]), f"ma{tag}")
                _lab(eng.tensor_mul(m[:, :, 1:40], t1[:, :, 1:40], v[:, :, 0:39]), f"mb{tag}")

            all_states = [(st, nc.vector) for st in dstates] + \
                         [(st, nc.gpsimd) for st in gstates]
            if act_interleave:
                na, nb = len(dstates), len(gstates)
                order = []
                for k in range(max(na, nb)):
                    if k < na:
                        order.append(all_states[k])
                    if k < nb:
                        order.append(all_states[na + k])
                act_states = order
            else:
                act_states = all_states

            # ---------------- time stepping ------------------------------
            # DVE chunk 0 (PE-assisted): TensorE accumulates its
            # A = sum_i (g_i a_i) m_i in PSUM via scaled-identity matmuls;
            # ACT extracts B = A + sum_i g_i b_i at stage 4 and the step
            # ends with one DVE add (s' = s + B).  Other chunks keep the
            # ACT-z path with the A-accumulation lagging a stage so
            # `A += z` never waits on ACT.  Pool w is self-served on Pool
            # (TensorScalarPtr).  ACT ops are chained in emission order.
            mult = mybir.AluOpType.mult
            add = mybir.AluOpType.add
            chain = _ActChain()

            def is_pe(st):
                return pe_assist and st is dstates[0]

            def interleave(states):
                na, nb = len(dstates), len(gstates)
                out = []
                for k in range(max(na, nb)):
                    if k < na:
                        out.append(states[k])
                    if k < nb:
                        out.append(states[na + k])
                return out

            rr = interleave(all_states) if act_interleave else list(all_states)
            t0 = 0.0
            for n in range(n_steps):
                dt = dts[n]
                cc = (dt / 2, dt / 2, dt)
                gg = (dt / 6, dt / 3, dt / 3, dt / 6)
                dorder = list(enumerate(dstates))
                for i in range(4):
                    ts = t0 + delta[i] * dt
                    a_i = math.exp(-ts)
                    b_i = F_FORCE * math.exp(ts)
                    # part 1: shifts, plain chunk first: the PE chunk's new
                    # s arrives via the ACT extract at the step boundary, so
                    # giving the plain chunk the head slot hides that.
                    for ci, st in dorder:
                        shifts_d(st, st["s"] if i == 0 else st["y"],
                                 f"_n{n}s{i}d{ci}")
                        if is_pe(st):
                            free = st["C"] * DIM
                            if i == 0:
                                # seed PSUM with s (unscaled identity), so
                                # the stage-4 extract yields s' directly
                                sf = st["s"][:, :, :].rearrange(
                                    "p d c -> p (d c)")
                                for k in range((free + 511) // 512):
                                    lo = k * 512
                                    hi = min(lo + 512, free)
                                    _lab(nc.tensor.matmul(
                                        psumA[:, lo:hi], ident[:, :],
                                        sf[:, lo:hi], start=True,
                                        stop=False), f"mmS_n{n}k{k}")
                            chain(_lab(nc.scalar.activation(
                                wtile[:, :], ident[:, :], Copy,
                                bias=0.0, scale=gg[i] * a_i),
                                f"Wscale_n{n}s{i}"))
                            mf = st["m"][:, :, :].rearrange("p d c -> p (d c)")
                            for k in range((free + 511) // 512):
                                lo, hi = k * 512, min((k + 1) * 512, free)
                                _lab(nc.tensor.matmul(
                                    psumA[:, lo:hi], wtile[:, :],
                                    mf[:, lo:hi],
                                    start=False, stop=(i == 3)),
                                    f"mm_n{n}s{i}k{k}")
                    for ci, st in enumerate(gstates):
                        shifts_g(st, st["s"] if i == 0 else st["y"],
                                 f"_n{n}s{i}g{ci}")
                    # pool w self-served on Pool: its y never waits on ACT
                    if i < 3 and pool_w_self:
                        for st in gstates:
                            _lab(nc.gpsimd.tensor_scalar(
                                st["w"][:, :, :], st["m"][:, :, :],
                                cc[i] * a_i, cc[i] * b_i, mult, add),
                                f"wTS_n{n}s{i}_{st['off']}")
                    # ACT w (critical path); the non-PE DVE chunk's w is
                    # split in dim-halves so its y can start earlier
                    if i < 3:
                        w_states = ([st for _, st in dorder]
                                    if pool_w_self else [s for s, _ in rr])
                        for st in w_states:
                            halves = ((slice(0, 20), slice(20, 40))
                                      if st in dstates and not is_pe(st)
                                      else (slice(0, DIM),))
                            for h, hs in enumerate(halves):
                                chain(_lab(nc.scalar.activation(
                                    st["w"][:, hs, :], st["m"][:, hs, :],
                                    Copy, bias=cc[i] * b_i,
                                    scale=cc[i] * a_i),
                                    f"w{h}_n{n}s{i}_{st['off']}"))
                    # y updates, then lagged A += z (late z must not block y)
                    if i < 3:
                        y_order = ([(st, nc.vector) for _, st in dorder]
                                   + [(st, nc.gpsimd) for st in gstates])
                        for st, eng in y_order:
                            if st in dstates and not is_pe(st):
                                for h, hs in enumerate(
                                        (slice(0, 20), slice(20, 40))):
                                    _lab(eng.tensor_add(
                                        st["y"][:, hs, :], st["s"][:, hs, :],
                                        st["w"][:, hs, :]),
                                        f"y{h}_n{n}s{i}_{st['off']}")
                            else:
                                _lab(eng.tensor_add(
                                    st["y"][:, :, :], st["s"][:, :, :],
                                    st["w"][:, :, :]),
                                    f"y_n{n}s{i}_{st['off']}")
                    if i >= 2:
                        for st, eng in all_states:
                            if is_pe(st):
                                continue
                            _lab(eng.tensor_add(
                                st["A"][:, :, :], st["A"][:, :, :],
                                st["z"][:, :, :]), f"Aadd_n{n}s{i}_{st['off']}")
                    # z affines for the non-PE chunks (consumed a stage
                    # later).  Stage-4 z of the plain DVE chunk runs on DVE
                    # (4x tensor_scalar) so the step-end s' never waits ACT.
                    for st, _ in rr:
                        if is_pe(st):
                            continue
                        if i == 3 and st in dstates:
                            _lab(nc.vector.tensor_scalar(
                                st["z"][:, :, :], st["m"][:, :, :],
                                gg[i] * a_i, gg[i] * b_i, mult, add),
                                f"zTS_n{n}s{i}_{st['off']}")
                            continue
                        zdst = st["A"] if i == 0 else st["z"]
                        chain(_lab(nc.scalar.activation(
                            zdst[:, :, :], st["m"][:, :, :], Copy,
                            bias=gg[i] * b_i, scale=gg[i] * a_i),
                            f"z_n{n}s{i}_{st['off']}"))
                    if i == 3 and pe_assist:
                        kbar = sum(
                            gg[j] * F_FORCE * math.exp(t0 + delta[j] * dt)
                            for j in range(4))
                        stp = dstates[0]
                        chain(_lab(nc.scalar.activation(
                            stp["y"][:, :, :].rearrange("p d c -> p (d c)"),
                            psumA[:, :], Copy, bias=kbar, scale=1.0),
                            f"Sex_n{n}"))
                # step end
                t0 += dt
                for st, eng in all_states:
                    if is_pe(st):
                        # s' was written into y by the PSUM extract
                        st["s"], st["y"] = st["y"], st["s"]
                    else:
                        _lab(eng.tensor_add(
                            st["y"][:, :, :], st["s"][:, :, :],
                            st["A"][:, :, :]), f"B_n{n}_{st['off']}")
                        _lab(eng.tensor_add(
                            st["s"][:, :, :], st["y"][:, :, :],
                            st["z"][:, :, :]), f"sfin_n{n}_{st['off']}")

            # ---------------- store ----------------------------------
            # Final state tiles go out as fp16 in their native layouts;
            # the host applies the e^-T unscale during unmarshalling.
            for j, st in enumerate(dstates):
                q = nc.sync if j == 0 else nc.scalar
                q.dma_start(xio[f"yd{j}"][:, :, :], st["s"][:, :, :])
            for j, st in enumerate(gstates):
                nc.scalar.dma_start(xio[f"yg{j}"][:, :, :], st["s"][:, :, :])

    nc.compile()
    return nc


def run(x: np.ndarray, trace: bool = False):
    """Run on the 8 cores; returns (output, BassKernelResults).

    The host marshals inputs/outputs: fp32 [BATCH, DIM] rows are split
    into per-chunk fp16 arrays laid out exactly like the device tiles
    (dim-major [P, DIM, C] for DVE chunks, row-major [P, C, DIM] for the
    Pool chunk), and the e^-T unscale of the integrating factor is applied
    on the way out.
    """
    import os

    from concourse.bass_utils import run_bass_kernel_spmd

    try:
        import antenv.axon_hooks  # noqa: F401
    except ImportError:
        # No NTFF hook in this image: tracing would crash on import, so
        # make sure an inherited BASS_TRACE can't switch it on.
        os.environ.setdefault("BASS_NEVER_TRACE", "1")
        trace = False

    if "nc" not in _CACHE:
        _CACHE["nc"] = build()
    nc = _CACHE["nc"]

    x = np.ascontiguousarray(np.asarray(x, dtype=np.float32))
    assert x.shape == (BATCH, DIM)
    shards = x.reshape(N_CORES, P, RB, DIM)

    chunks = []  # (name, offset, C, dim_major)
    off = 0
    for j, C in enumerate(DVE_CHUNKS):
        chunks.append((f"d{j}", off, C, True))
        off += C
    for j, C in enumerate(GP_CHUNKS):
        chunks.append((f"g{j}", off, C, False))
        off += C
    assert off == RB

    in_maps = []
    for i in range(N_CORES):
        m = {}
        for name, o, C, dim_major in chunks:
            part = shards[i, :, o:o + C, :].astype(np.float16)
            if dim_major:
                part = np.ascontiguousarray(part.transpose(0, 2, 1))
            m[f"x{name}"] = np.ascontiguousarray(part)
        in_maps.append(m)

    res = run_bass_kernel_spmd(nc, in_maps, list(range(N_CORES)), trace=trace)

    out = np.empty((N_CORES, P, RB, DIM), dtype=np.float32)
    scale = np.float32(math.exp(-T_END))
    for i in range(N_CORES):
        r = res.results[i]
        for name, o, C, dim_major in chunks:
            part = r[f"y{name}"].astype(np.float32)
            if dim_major:
                part = part.transpose(0, 2, 1)
            out[i, :, o:o + C, :] = part * scale
    return out.reshape(BATCH, DIM), res


def kernel(x: np.ndarray) -> np.ndarray:
    return run(x)[0]


# revision 45
# speedup vs baseline: 1.0396x; 1.0396x over previous
"""Lorenz96 RK4 integrator on TRN2 — 8-core data parallel Bass kernel (v3).

Math: integrate dx_i/dt = (x_{i+1} - x_{i-2}) * x_{i-1} - x_i + F (cyclic,
F=8) from t=0 to t=1 for 262144 independent trajectories of dim 40.

Numerics
- Integrating factor s = e^t x: the ODE becomes ds/dt = a(t)*N(s) + b(t)
  with N(s) = (roll(s,-1)-roll(s,2))*roll(s,1) (degree-2 homogeneous),
  a = e^-t, b = F e^t.  The "- x + F" part of the derivative turns into
  per-stage compile-time scalars that ride free scale/bias slots, so a
  classic RK4 step needs only 15 tensor-tensor passes per element (vs 19
  for the direct form), and fewer for the PE-assisted chunk below.
- fp16 state: plain tensor_tensor gets the DVE 2x_1p perf mode for 2-byte
  dtypes.  fp16 noise is ~1e-3 at N=10 (truncation dominates).
- N_STEPS=10 with a geometric step schedule dt_n ~ 0.96^n: late-step local
  error dominates here, so slightly shrinking late steps buys accuracy for
  free (device-measured scaled max rel err 1.43e-2 vs the 2e-2 gate;
  uniform N=10 would be 2.5e-2, uniform N=12 1.13e-2).

Mapping (per core: 32768 rows = 128 partitions x 256 row-blocks)
- DVE chunk 0 (102 rb, fp16, dim-major [P,40,C]): shifts are slices along
  the middle axis, keeping every operand innermost-contiguous and 4B
  aligned (C even) for the 2x mode.  Its z-accumulation A = sum g_i a_i m_i
  runs on the otherwise-idle TensorE as scaled-identity matmuls
  accumulating in PSUM, seeded with s at stage 1, so ACT's stage-4 extract
  (+ sum g_i b_i bias) IS the new state: this chunk costs DVE only 11
  passes/step (8 shift + 3 y).
- DVE chunk 1 (110 rb): same layout, ACT computes its w/z affines (the
  A-accumulation lags a stage so `A += z` never waits on ACT; stage-4's z
  rides a 4x DVE tensor_scalar so the step-end never waits either).
- Pool chunk (44 rb, fp16, row-major): GpSimd runs the same 15 passes with
  its w affine self-served as a Pool TensorScalarPtr; ACT does its z's.
- ACT ops are chained in emission order with ordering-only deps: Tile
  schedules each engine with its own internal cost model, and without the
  chain a pacing mismatch lets one path's affines pile up ahead of the
  other's in ACT's static order (observed: paths drifting 4 steps apart,
  ~50us stalls).
- Host-marshalled I/O: run() converts to fp16 and pre-lays chunks out in
  tile layout, so the device does no conversion passes and DMA bytes are
  halved; the e^-T unscale happens on the host side too.

Engine busy per step (TimelineSim): DVE ~62us, Pool ~63us, ACT ~46us,
PE ~27us; span 652us/core vs the 1963us v1 baseline (3.0x).
"""

import math

import numpy as np

F_FORCE = 8.0
T_END = 1.0
BATCH, DIM = 262144, 40
N_CORES = 8
ROWS = BATCH // N_CORES  # rows per core
P = 128                  # SBUF partitions
RB = ROWS // P           # row-blocks per partition (256)

N_STEPS = 10
RHO = 0.96  # geometric step-size ratio (dt_n ~ RHO^n, normalized to sum 1)
DT = T_END / N_STEPS

# rows-per-partition chunk sizes (sum must equal RB); keep C even so the
# fp16 dim-slices stay 4B-aligned.  DVE_CHUNKS[0] is the PE-assisted chunk
# (its A accumulator must fit PSUM: C*40 <= 4096 fp32).
DVE_CHUNKS = (102, 110)   # fp16 dim-major chunks owned by the Vector engine
GP_CHUNKS = (44,)         # fp16 row-major chunk owned by the Pool engine

_CACHE: dict = {}
LABELS: dict = {}  # instruction name -> human label (diagnostics)


def _lab(inst, label):
    try:
        LABELS[inst.ins.name] = label
    except Exception:
        pass
    return inst


class _ActChain:
    """Force the Tile scheduler to keep ACT instructions in emission order
    via ordering-only (no-sync) dependencies.  Tile schedules each engine's
    static order with its own internal cost model; when two independent
    compute paths share ACT, a pacing mismatch lets one path's affine ops
    pile up ahead of the other's in the static order, which then starves
    the other path at runtime (observed: paths drifting 4 steps apart and
    ~50us stalls).  Chaining pins the order so both paths stay in lockstep.
    """

    def __init__(self):
        self.last = None

    def __call__(self, inst):
        from concourse.instruction_name_ordered_set import (
            InstructionNameOrderedSet,
        )
        if self.last is not None:
            s = InstructionNameOrderedSet()
            s.add(self.last)
            inst.ins.add_nosync_dependencies_from(s)
        self.last = inst.ins.name
        return inst


def build(n_steps=N_STEPS, rows=ROWS, dve_chunks=DVE_CHUNKS,
          gp_chunks=GP_CHUNKS, rho=RHO, act_interleave=True,
          pool_w_self=True, pe_assist=True):
    """Build the Bass module for one core's shard.

    pe_assist: run the first DVE chunk's z-accumulation on TensorE/PSUM.
    pool_w_self: Pool computes its own w affine (TensorScalarPtr) instead
      of depending on ACT for its y critical path.
    act_interleave: order ACT's per-stage ops DVE/Pool interleaved.
    """
    import concourse.mybir as mybir
    from concourse import bacc, bass, tile
    from concourse.masks import make_identity

    f16 = mybir.dt.float16
    f32 = mybir.dt.float32
    Copy = mybir.ActivationFunctionType.Copy

    rb = rows // P
    assert sum(dve_chunks) + sum(gp_chunks) == rb
    assert all(C % 2 == 0 for C in dve_chunks)

    # Geometric step schedule: dt_n ~ rho^n (sum = T_END).  Late-step local
    # error dominates the final error for this system, so rho slightly
    # below 1 (late steps smaller) buys accuracy for free.
    wts = [rho ** k for k in range(n_steps)]
    dts = [T_END * w / sum(wts) for w in wts]

    # RK4 stage constants (classic): y2 = s + (dt/2)k1, y3 = s + (dt/2)k2,
    # y4 = s + dt*k3, s' = s + sum(g_i k_i); k_i = a_i*m_i + b_i in s-space.
    delta = (0.0, 0.5, 0.5, 1.0)

    nc = bacc.Bacc("TRN2", target_bir_lowering=False, debug=False)
    # Host-marshalled I/O: the host converts to fp16 and lays each chunk
    # out exactly as its SBUF tile (dim-major [P,DIM,C] for DVE chunks,
    # row-major [P,C,DIM] for Pool chunks), so the device runs no
    # conversion passes and DMA bytes are halved.
    xio = {}
    for j, C in enumerate(dve_chunks):
        xio[f"xd{j}"] = nc.dram_tensor(f"xd{j}", [P, DIM, C], f16,
                                       kind="ExternalInput")
        xio[f"yd{j}"] = nc.dram_tensor(f"yd{j}", [P, DIM, C], f16,
                                       kind="ExternalOutput")
    for j, C in enumerate(gp_chunks):
        xio[f"xg{j}"] = nc.dram_tensor(f"xg{j}", [P, C, DIM], f16,
                                       kind="ExternalInput")
        xio[f"yg{j}"] = nc.dram_tensor(f"yg{j}", [P, C, DIM], f16,
                                       kind="ExternalOutput")

    with tile.TileContext(nc) as tc:
        with tc.tile_pool(name="work", bufs=1) as pool, \
             tc.tile_pool(name="acc", space=bass.MemorySpace.PSUM,
                          bufs=1) as ppool:

            # ---------------- allocate chunks, issue input DMAs ----------
            off = 0
            gstates = []
            for j, C in enumerate(gp_chunks):
                s = {
                    "C": C, "off": off, "j": f"g{j}", "io": f"g{j}",
                    # s gets the DMA directly (fp16 row-major state)
                    "s": pool.tile([P, C, DIM], f16, tag=f"s_g{j}",
                                   name=f"s_g{j}"),
                    "y": pool.tile([P, C, DIM], f16, tag=f"y_g{j}",
                                   name=f"y_g{j}"),
                    "t1": pool.tile([P, C, DIM], f16, tag=f"t1_g{j}",
                                    name=f"t1_g{j}"),
                    "w": pool.tile([P, C, DIM], f16, tag=f"w_g{j}",
                                   name=f"w_g{j}"),
                    "A": pool.tile([P, C, DIM], f16, tag=f"A_g{j}",
                                   name=f"A_g{j}"),
                    "z": pool.tile([P, C, DIM], f16, tag=f"z_g{j}",
                                   name=f"z_g{j}"),
                }
                gstates.append(s)
                off += C
            # PE-assist machinery: the first DVE chunk's z-accumulation
            # A = sum_i (g_i a_i) m_i runs on the otherwise-idle TensorE as
            # scaled-identity matmuls accumulating into PSUM; ACT extracts
            # B = A + sum_i g_i b_i.  PSUM (16 KiB/partition = 4096 fp32)
            # fits one C=102 chunk (4080 fp32).
            ident = wtile = psumA = None
            if pe_assist:
                assert dve_chunks and dve_chunks[0] * DIM <= 4096
                ident = pool.tile([P, P], f16, tag="ident", name="ident")
                wtile = pool.tile([P, P], f16, tag="W", bufs=2, name="W")
                psumA = ppool.tile([P, dve_chunks[0] * DIM], f32, tag="A_pe",
                                   name="A_pe")
            dstates = []
            for j, C in enumerate(dve_chunks):
                pe = pe_assist and j == 0
                s = {
                    "C": C, "off": off, "j": j, "io": f"d{j}",
                    "s": pool.tile([P, DIM, C], f16, tag=f"s_d{j}",
                                   name=f"s_d{j}"),
                    "y": pool.tile([P, DIM, C], f16, tag=f"y_d{j}",
                                   name=f"y_d{j}"),
                    "t1": pool.tile([P, DIM, C], f16, tag=f"t1_d{j}",
                                    name=f"t1_d{j}"),
                    "w": pool.tile([P, DIM, C], f16, tag=f"w_d{j}",
                                   name=f"w_d{j}"),
                }
                if not pe:
                    # the PE chunk accumulates in PSUM: no A/z tiles
                    s["A"] = pool.tile([P, DIM, C], f16, tag=f"A_d{j}",
                                       name=f"A_d{j}")
                    s["z"] = pool.tile([P, DIM, C], f16, tag=f"z_d{j}",
                                       name=f"z_d{j}")
                dstates.append(s)
                off += C

            def fresh_m(st, dim_major):
                # rotate the m tile per stage (bufs=2): the next stage's
                # shift write never waits on ACT's z still reading the
                # previous m.  The PE chunk has no ACT z reader, so a
                # single buffer suffices there.
                j = st.get("j", st["off"])
                nb = 1 if (pe_assist and dstates and st is dstates[0]) else 2
                shape = [P, DIM, st["C"]] if dim_major else [P, st["C"], DIM]
                st["m"] = pool.tile(shape, f16, tag=f"m_{dim_major}_{j}",
                                    bufs=nb, name=f"m_{j}")
                return st["m"]

            # Input DMAs: straight into the state tiles (host already
            # converted and transposed).  Spread across both HWDGE queues.
            nc.sync.dma_start(dstates[0]["s"][:, :, :], xio["xd0"][:, :, :])
            for j, g in enumerate(gstates):
                nc.scalar.dma_start(g["s"][:, :, :], xio[f"xg{j}"][:, :, :])
            for j, d in enumerate(dstates[1:], start=1):
                nc.sync.dma_start(d["s"][:, :, :], xio[f"xd{j}"][:, :, :])

            if pe_assist:
                make_identity(nc, ident[:, :])

            # ---------------- shift helpers ------------------------------
            def shifts_d(st, v, tag=""):
                # dim-major fp16: slices along the middle (dim) axis.
                t1, m = st["t1"], fresh_m(st, True)
                eng = nc.vector
                # t1 = roll(v,-1) - roll(v,2)
                _lab(eng.tensor_sub(t1[:, 0:2, :], v[:, 1:3, :], v[:, 38:40, :]), f"t1a{tag}")
                _lab(eng.tensor_sub(t1[:, 2:39, :], v[:, 3:40, :], v[:, 0:37, :]), f"t1b{tag}")
                _lab(eng.tensor_sub(t1[:, 39:40, :], v[:, 0:1, :], v[:, 37:38, :]), f"t1c{tag}")
                # m = t1 * roll(v,1)
                _lab(eng.tensor_mul(m[:, 0:1, :], t1[:, 0:1, :], v[:, 39:40, :]), f"ma{tag}")
                _lab(eng.tensor_mul(m[:, 1:40, :], t1[:, 1:40, :], v[:, 0:39, :]), f"mb{tag}")

            def shifts_g(st, v, tag=""):
                # row-major fp32: slices along the last (dim) axis.
                t1, m = st["t1"], fresh_m(st, False)
                eng = nc.gpsimd
                _lab(eng.tensor_sub(t1[:, :, 0:2], v[:, :, 1:3], v[:, :, 38:40]), f"t1a{tag}")
                _lab(eng.tensor_sub(t1[:, :, 2:39], v[:, :, 3:40], v[:, :, 0:37]), f"t1b{tag}")
                _lab(eng.tensor_sub(t1[:, :, 39:40], v[:, :, 0:1], v[:, :, 37:38]), f"t1c{tag}")
                _lab(eng.tensor_mul(m[:, :, 0:1], t1[:, :, 0:1], v[:, :, 39:40]), f"ma{tag}")
                _lab(eng.tensor_mul(m[:, :, 1:40], t1[:, :, 1:40], v[:, :, 0:39]), f"mb{tag}")

            all_states = [(st, nc.vector) for st in dstates] + \
                         [(st, nc.gpsimd) for st in gstates]
            if act_interleave:
                na, nb = len(dstates), len(gstates)
                order = []
                for k in range(max(na, nb)):
                    if k < na:
                        order.append(all_states[k])
                    if k < nb:
                        order.append(all_states[na + k])
                act_states = order
            else:
                act_states = all_states

            # ---------------- time stepping ------------------------------
            # DVE chunk 0 (PE-assisted): TensorE accumulates its
            # A = sum_i (g_i a_i) m_i in PSUM via scaled-identity matmuls;
            # ACT extracts B = A + sum_i g_i b_i at stage 4 and the step
            # ends with one DVE add (s' = s + B).  Other chunks keep the
            # ACT-z path with the A-accumulation lagging a stage so
            # `A += z` never waits on ACT.  Pool w is self-served on Pool
            # (TensorScalarPtr).  ACT ops are chained in emission order.
            mult = mybir.AluOpType.mult
            add = mybir.AluOpType.add
            chain = _ActChain()

            def is_pe(st):
                return pe_assist and st is dstates[0]

            def interleave(states):
                na, nb = len(dstates), len(gstates)
                out = []
                for k in range(max(na, nb)):
                    if k < na:
                        out.append(states[k])
                    if k < nb:
                        out.append(states[na + k])
                return out

            rr = interleave(all_states) if act_interleave else list(all_states)
            t0 = 0.0
            for n in range(n_steps):
                dt = dts[n]
                cc = (dt / 2, dt / 2, dt)
                gg = (dt / 6, dt / 3, dt / 3, dt / 6)
                dorder = list(enumerate(dstates))
                for i in range(4):
                    ts = t0 + delta[i] * dt
                    a_i = math.exp(-ts)
                    b_i = F_FORCE * math.exp(ts)
                    # part 1: shifts, plain chunk first: the PE chunk's new
                    # s arrives via the ACT extract at the step boundary, so
                    # giving the plain chunk the head slot hides that.
                    for ci, st in dorder:
                        shifts_d(st, st["s"] if i == 0 else st["y"],
                                 f"_n{n}s{i}d{ci}")
                        if is_pe(st):
                            free = st["C"] * DIM
                            if i == 0:
                                # seed PSUM with s (unscaled identity), so
                                # the stage-4 extract yields s' directly
                                sf = st["s"][:, :, :].rearrange(
                                    "p d c -> p (d c)")
                                for k in range((free + 511) // 512):
                                    lo = k * 512
                                    hi = min(lo + 512, free)
                                    _lab(nc.tensor.matmul(
                                        psumA[:, lo:hi], ident[:, :],
                                        sf[:, lo:hi], start=True,
                                        stop=False), f"mmS_n{n}k{k}")
                            chain(_lab(nc.scalar.activation(
                                wtile[:, :], ident[:, :], Copy,
                                bias=0.0, scale=gg[i] * a_i),
                                f"Wscale_n{n}s{i}"))
                            mf = st["m"][:, :, :].rearrange("p d c -> p (d c)")
                            for k in range((free + 511) // 512):
                                lo, hi = k * 512, min((k + 1) * 512, free)
                                _lab(nc.tensor.matmul(
                                    psumA[:, lo:hi], wtile[:, :],
                                    mf[:, lo:hi],
                                    start=False, stop=(i == 3)),
                                    f"mm_n{n}s{i}k{k}")
                    for ci, st in enumerate(gstates):
                        shifts_g(st, st["s"] if i == 0 else st["y"],
                                 f"_n{n}s{i}g{ci}")
                    # pool w self-served on Pool: its y never waits on ACT
                    if i < 3 and pool_w_self:
                        for st in gstates:
                            _lab(nc.gpsimd.tensor_scalar(
                                st["w"][:, :, :], st["m"][:, :, :],
                                cc[i] * a_i, cc[i] * b_i, mult, add),
                                f"wTS_n{n}s{i}_{st['off']}")
                    # ACT w (critical path); the non-PE DVE chunk's w is
                    # split in dim-halves so its y can start earlier
                    if i < 3:
                        w_states = ([st for _, st in dorder]
                                    if pool_w_self else [s for s, _ in rr])
                        for st in w_states:
                            chain(_lab(nc.scalar.activation(
                                st["w"][:, :, :], st["m"][:, :, :],
                                Copy, bias=cc[i] * b_i,
                                scale=cc[i] * a_i),
                                f"w_n{n}s{i}_{st['off']}"))
                    # y updates, then lagged A += z (late z must not block y)
                    if i < 3:
                        y_order = ([(st, nc.vector) for _, st in dorder]
                                   + [(st, nc.gpsimd) for st in gstates])
                        for st, eng in y_order:
                            _lab(eng.tensor_add(
                                st["y"][:, :, :], st["s"][:, :, :],
                                st["w"][:, :, :]), f"y_n{n}s{i}_{st['off']}")
                    if i >= 2:
                        for st, eng in all_states:
                            if is_pe(st):
                                continue
                            _lab(eng.tensor_add(
                                st["A"][:, :, :], st["A"][:, :, :],
                                st["z"][:, :, :]), f"Aadd_n{n}s{i}_{st['off']}")
                    # z affines for the non-PE chunks (consumed a stage
                    # later).  Stage-4 z of the plain DVE chunk runs on DVE
                    # (4x tensor_scalar) so the step-end s' never waits ACT.
                    for st, _ in rr:
                        if is_pe(st):
                            continue
                        if i == 3 and st in dstates:
                            _lab(nc.vector.tensor_scalar(
                                st["z"][:, :, :], st["m"][:, :, :],
                                gg[i] * a_i, gg[i] * b_i, mult, add),
                                f"zTS_n{n}s{i}_{st['off']}")
                            continue
                        zdst = st["A"] if i == 0 else st["z"]
                        chain(_lab(nc.scalar.activation(
                            zdst[:, :, :], st["m"][:, :, :], Copy,
                            bias=gg[i] * b_i, scale=gg[i] * a_i),
                            f"z_n{n}s{i}_{st['off']}"))
                    if i == 3 and pe_assist:
                        kbar = sum(
                            gg[j] * F_FORCE * math.exp(t0 + delta[j] * dt)
                            for j in range(4))
                        stp = dstates[0]
                        chain(_lab(nc.scalar.activation(
                            stp["y"][:, :, :].rearrange("p d c -> p (d c)"),
                            psumA[:, :], Copy, bias=kbar, scale=1.0),
                            f"Sex_n{n}"))
                # step end
                t0 += dt
                for st, eng in all_states:
                    if is_pe(st):
                        # s' was written into y by the PSUM extract
                        st["s"], st["y"] = st["y"], st["s"]
                    else:
                        _lab(eng.tensor_add(
                            st["y"][:, :, :], st["s"][:, :, :],
                            st["A"][:, :, :]), f"B_n{n}_{st['off']}")
                        _lab(eng.tensor_add(
                            st["s"][:, :, :], st["y"][:, :, :],
                            st["z"][:, :, :]), f"sfin_n{n}_{st['off']}")

            # ---------------- store ----------------------------------
            # Final state tiles go out as fp16 in their native layouts;
            # the host applies the e^-T unscale during unmarshalling.
            for j, st in enumerate(dstates):
                q = nc.sync if j == 0 else nc.scalar
                q.dma_start(xio[f"yd{j}"][:, :, :], st["s"][:, :, :])
            for j, st in enumerate(gstates):
                nc.scalar.dma_start(xio[f"yg{j}"][:, :, :], st["s"][:, :, :])

    nc.compile()
    return nc


def run(x: np.ndarray, trace: bool = False):
    """Run on the 8 cores; returns (output, BassKernelResults).

    The host marshals inputs/outputs: fp32 [BATCH, DIM] rows are split
    into per-chunk fp16 arrays laid out exactly like the device tiles
    (dim-major [P, DIM, C] for DVE chunks, row-major [P, C, DIM] for the
    Pool chunk), and the e^-T unscale of the integrating factor is applied
    on the way out.
    """
    import os

    from concourse.bass_utils import run_bass_kernel_spmd

    try:
        import antenv.axon_hooks  # noqa: F401
    except ImportError:
        # No NTFF hook in this image: tracing would crash on import, so
        # make sure an inherited BASS_TRACE can't switch it on.
        os.environ.setdefault("BASS_NEVER_TRACE", "1")
        trace = False

    if "nc" not in _CACHE:
        _CACHE["nc"] = build()
    nc = _CACHE["nc"]

    x = np.ascontiguousarray(np.asarray(x, dtype=np.float32))
    assert x.shape == (BATCH, DIM)
    shards = x.reshape(N_CORES, P, RB, DIM)

    chunks = []  # (name, offset, C, dim_major)
    off = 0
    for j, C in enumerate(DVE_CHUNKS):
        chunks.append((f"d{j}", off, C, True))
        off += C
    for j, C in enumerate(GP_CHUNKS):
        chunks.append((f"g{j}", off, C, False))
        off += C
    assert off == RB

    in_maps = []
    for i in range(N_CORES):
        m = {}
        for name, o, C, dim_major in chunks:
            part = shards[i, :, o:o + C, :].astype(np.float16)
            if dim_major:
                part = np.ascontiguousarray(part.transpose(0, 2, 1))
            m[f"x{name}"] = np.ascontiguousarray(part)
        in_maps.append(m)

    res = run_bass_kernel_spmd(nc, in_maps, list(range(N_CORES)), trace=trace)

    out = np.empty((N_CORES, P, RB, DIM), dtype=np.float32)
    scale = np.float32(math.exp(-T_END))
    for i in range(N_CORES):
        r = res.results[i]
        for name, o, C, dim_major in chunks:
            part = r[f"y{name}"].astype(np.float32)
            if dim_major:
                part = part.transpose(0, 2, 1)
            out[i, :, o:o + C, :] = part * scale
    return out.reshape(BATCH, DIM), res


def kernel(x: np.ndarray) -> np.ndarray:
    return run(x)[0]


# revision 46
# speedup vs baseline: 1.0576x; 1.0173x over previous
"""Lorenz96 RK4 integrator on TRN2 — 8-core data parallel Bass kernel (v3).

Math: integrate dx_i/dt = (x_{i+1} - x_{i-2}) * x_{i-1} - x_i + F (cyclic,
F=8) from t=0 to t=1 for 262144 independent trajectories of dim 40.

Numerics
- Integrating factor s = e^t x: the ODE becomes ds/dt = a(t)*N(s) + b(t)
  with N(s) = (roll(s,-1)-roll(s,2))*roll(s,1) (degree-2 homogeneous),
  a = e^-t, b = F e^t.  The "- x + F" part of the derivative turns into
  per-stage compile-time scalars that ride free scale/bias slots, so a
  classic RK4 step needs only 15 tensor-tensor passes per element (vs 19
  for the direct form), and fewer for the PE-assisted chunk below.
- fp16 state: plain tensor_tensor gets the DVE 2x_1p perf mode for 2-byte
  dtypes.  fp16 noise is ~1e-3 at N=10 (truncation dominates).
- N_STEPS=10 with a geometric step schedule dt_n ~ 0.96^n: late-step local
  error dominates here, so slightly shrinking late steps buys accuracy for
  free (device-measured scaled max rel err 1.43e-2 vs the 2e-2 gate;
  uniform N=10 would be 2.5e-2, uniform N=12 1.13e-2).

Mapping (per core: 32768 rows = 128 partitions x 256 row-blocks)
- DVE chunk 0 (102 rb, fp16, dim-major [P,40,C]): shifts are slices along
  the middle axis, keeping every operand innermost-contiguous and 4B
  aligned (C even) for the 2x mode.  Its z-accumulation A = sum g_i a_i m_i
  runs on the otherwise-idle TensorE as scaled-identity matmuls
  accumulating in PSUM, seeded with s at stage 1, so ACT's stage-4 extract
  (+ sum g_i b_i bias) IS the new state: this chunk costs DVE only 11
  passes/step (8 shift + 3 y).
- DVE chunk 1 (110 rb): same layout, ACT computes its w/z affines (the
  A-accumulation lags a stage so `A += z` never waits on ACT; stage-4's z
  rides a 4x DVE tensor_scalar so the step-end never waits either).
- Pool chunk (44 rb, fp16, row-major): GpSimd runs the same 15 passes with
  its w affine self-served as a Pool TensorScalarPtr; ACT does its z's.
- ACT ops are chained in emission order with ordering-only deps: Tile
  schedules each engine with its own internal cost model, and without the
  chain a pacing mismatch lets one path's affines pile up ahead of the
  other's in ACT's static order (observed: paths drifting 4 steps apart,
  ~50us stalls).
- Host-marshalled I/O: run() converts to fp16 and pre-lays chunks out in
  tile layout, so the device does no conversion passes and DMA bytes are
  halved; the e^-T unscale happens on the host side too.

Engine busy per step (TimelineSim): DVE ~62us, Pool ~63us, ACT ~46us,
PE ~27us; span 652us/core vs the 1963us v1 baseline (3.0x).
"""

import math

import numpy as np

F_FORCE = 8.0
T_END = 1.0
BATCH, DIM = 262144, 40
N_CORES = 8
ROWS = BATCH // N_CORES  # rows per core
P = 128                  # SBUF partitions
RB = ROWS // P           # row-blocks per partition (256)

N_STEPS = 10
RHO = 0.96  # geometric step-size ratio (dt_n ~ RHO^n, normalized to sum 1)
DT = T_END / N_STEPS

# rows-per-partition chunk sizes (sum must equal RB); keep C even so the
# fp16 dim-slices stay 4B-aligned.  DVE_CHUNKS[0] is the PE-assisted chunk
# (its A accumulator must fit PSUM: C*40 <= 4096 fp32).
DVE_CHUNKS = (102, 110)   # fp16 dim-major chunks owned by the Vector engine
GP_CHUNKS = (44,)         # fp16 row-major chunk owned by the Pool engine

_CACHE: dict = {}
LABELS: dict = {}  # instruction name -> human label (diagnostics)


def _lab(inst, label):
    try:
        LABELS[inst.ins.name] = label
    except Exception:
        pass
    return inst


class _ActChain:
    """Force the Tile scheduler to keep ACT instructions in emission order
    via ordering-only (no-sync) dependencies.  Tile schedules each engine's
    static order with its own internal cost model; when two independent
    compute paths share ACT, a pacing mismatch lets one path's affine ops
    pile up ahead of the other's in the static order, which then starves
    the other path at runtime (observed: paths drifting 4 steps apart and
    ~50us stalls).  Chaining pins the order so both paths stay in lockstep.
    """

    def __init__(self):
        self.last = None

    def __call__(self, inst):
        from concourse.instruction_name_ordered_set import (
            InstructionNameOrderedSet,
        )
        if self.last is not None:
            s = InstructionNameOrderedSet()
            s.add(self.last)
            inst.ins.add_nosync_dependencies_from(s)
        self.last = inst.ins.name
        return inst


def build(n_steps=N_STEPS, rows=ROWS, dve_chunks=DVE_CHUNKS,
          gp_chunks=GP_CHUNKS, rho=RHO, act_interleave=True,
          pool_w_self=False, pe_assist=True):
    """Build the Bass module for one core's shard.

    pe_assist: run the first DVE chunk's z-accumulation on TensorE/PSUM.
    pool_w_self: Pool computes its own w affine (TensorScalarPtr) instead
      of depending on ACT for its y critical path.
    act_interleave: order ACT's per-stage ops DVE/Pool interleaved.
    """
    import concourse.mybir as mybir
    from concourse import bacc, bass, tile
    from concourse.masks import make_identity

    f16 = mybir.dt.float16
    f32 = mybir.dt.float32
    Copy = mybir.ActivationFunctionType.Copy

    rb = rows // P
    assert sum(dve_chunks) + sum(gp_chunks) == rb
    assert all(C % 2 == 0 for C in dve_chunks)

    # Geometric step schedule: dt_n ~ rho^n (sum = T_END).  Late-step local
    # error dominates the final error for this system, so rho slightly
    # below 1 (late steps smaller) buys accuracy for free.
    wts = [rho ** k for k in range(n_steps)]
    dts = [T_END * w / sum(wts) for w in wts]

    # RK4 stage constants (classic): y2 = s + (dt/2)k1, y3 = s + (dt/2)k2,
    # y4 = s + dt*k3, s' = s + sum(g_i k_i); k_i = a_i*m_i + b_i in s-space.
    delta = (0.0, 0.5, 0.5, 1.0)

    nc = bacc.Bacc("TRN2", target_bir_lowering=False, debug=False)
    # Host-marshalled I/O: the host converts to fp16 and lays each chunk
    # out exactly as its SBUF tile (dim-major [P,DIM,C] for DVE chunks,
    # row-major [P,C,DIM] for Pool chunks), so the device runs no
    # conversion passes and DMA bytes are halved.
    xio = {}
    for j, C in enumerate(dve_chunks):
        xio[f"xd{j}"] = nc.dram_tensor(f"xd{j}", [P, DIM, C], f16,
                                       kind="ExternalInput")
        xio[f"yd{j}"] = nc.dram_tensor(f"yd{j}", [P, DIM, C], f16,
                                       kind="ExternalOutput")
    for j, C in enumerate(gp_chunks):
        xio[f"xg{j}"] = nc.dram_tensor(f"xg{j}", [P, C, DIM], f16,
                                       kind="ExternalInput")
        xio[f"yg{j}"] = nc.dram_tensor(f"yg{j}", [P, C, DIM], f16,
                                       kind="ExternalOutput")

    with tile.TileContext(nc) as tc:
        with tc.tile_pool(name="work", bufs=1) as pool, \
             tc.tile_pool(name="acc", space=bass.MemorySpace.PSUM,
                          bufs=1) as ppool:

            # ---------------- allocate chunks, issue input DMAs ----------
            off = 0
            gstates = []
            for j, C in enumerate(gp_chunks):
                s = {
                    "C": C, "off": off, "j": f"g{j}", "io": f"g{j}",
                    # s gets the DMA directly (fp16 row-major state)
                    "s": pool.tile([P, C, DIM], f16, tag=f"s_g{j}",
                                   name=f"s_g{j}"),
                    "y": pool.tile([P, C, DIM], f16, tag=f"y_g{j}",
                                   name=f"y_g{j}"),
                    "t1": pool.tile([P, C, DIM], f16, tag=f"t1_g{j}",
                                    name=f"t1_g{j}"),
                    "w": pool.tile([P, C, DIM], f16, tag=f"w_g{j}",
                                   name=f"w_g{j}"),
                    "A": pool.tile([P, C, DIM], f16, tag=f"A_g{j}",
                                   name=f"A_g{j}"),
                    "z": pool.tile([P, C, DIM], f16, tag=f"z_g{j}",
                                   name=f"z_g{j}"),
                }
                gstates.append(s)
                off += C
            # PE-assist machinery: the first DVE chunk's z-accumulation
            # A = sum_i (g_i a_i) m_i runs on the otherwise-idle TensorE as
            # scaled-identity matmuls accumulating into PSUM; ACT extracts
            # B = A + sum_i g_i b_i.  PSUM (16 KiB/partition = 4096 fp32)
            # fits one C=102 chunk (4080 fp32).
            ident = wtile = psumA = None
            if pe_assist:
                assert dve_chunks and dve_chunks[0] * DIM <= 4096
                ident = pool.tile([P, P], f16, tag="ident", name="ident")
                wtile = pool.tile([P, P], f16, tag="W", bufs=2, name="W")
                psumA = ppool.tile([P, dve_chunks[0] * DIM], f32, tag="A_pe",
                                   name="A_pe")
            dstates = []
            for j, C in enumerate(dve_chunks):
                pe = pe_assist and j == 0
                s = {
                    "C": C, "off": off, "j": j, "io": f"d{j}",
                    "s": pool.tile([P, DIM, C], f16, tag=f"s_d{j}",
                                   name=f"s_d{j}"),
                    "y": pool.tile([P, DIM, C], f16, tag=f"y_d{j}",
                                   name=f"y_d{j}"),
                    "t1": pool.tile([P, DIM, C], f16, tag=f"t1_d{j}",
                                    name=f"t1_d{j}"),
                    "w": pool.tile([P, DIM, C], f16, tag=f"w_d{j}",
                                   name=f"w_d{j}"),
                }
                if not pe:
                    # the PE chunk accumulates in PSUM: no A/z tiles
                    s["A"] = pool.tile([P, DIM, C], f16, tag=f"A_d{j}",
                                       name=f"A_d{j}")
                    s["z"] = pool.tile([P, DIM, C], f16, tag=f"z_d{j}",
                                       name=f"z_d{j}")
                dstates.append(s)
                off += C

            def fresh_m(st, dim_major):
                # rotate the m tile per stage (bufs=2): the next stage's
                # shift write never waits on ACT's z still reading the
                # previous m.  The PE chunk has no ACT z reader, so a
                # single buffer suffices there.
                j = st.get("j", st["off"])
                nb = 1 if (pe_assist and dstates and st is dstates[0]) else 2
                shape = [P, DIM, st["C"]] if dim_major else [P, st["C"], DIM]
                st["m"] = pool.tile(shape, f16, tag=f"m_{dim_major}_{j}",
                                    bufs=nb, name=f"m_{j}")
                return st["m"]

            # Input DMAs: straight into the state tiles (host already
            # converted and transposed).  Spread across both HWDGE queues.
            nc.sync.dma_start(dstates[0]["s"][:, :, :], xio["xd0"][:, :, :])
            for j, g in enumerate(gstates):
                nc.scalar.dma_start(g["s"][:, :, :], xio[f"xg{j}"][:, :, :])
            for j, d in enumerate(dstates[1:], start=1):
                nc.sync.dma_start(d["s"][:, :, :], xio[f"xd{j}"][:, :, :])

            if pe_assist:
                make_identity(nc, ident[:, :])

            # ---------------- shift helpers ------------------------------
            def shifts_d(st, v, tag=""):
                # dim-major fp16: slices along the middle (dim) axis.
                t1, m = st["t1"], fresh_m(st, True)
                eng = nc.vector
                # t1 = roll(v,-1) - roll(v,2)
                _lab(eng.tensor_sub(t1[:, 0:2, :], v[:, 1:3, :], v[:, 38:40, :]), f"t1a{tag}")
                _lab(eng.tensor_sub(t1[:, 2:39, :], v[:, 3:40, :], v[:, 0:37, :]), f"t1b{tag}")
                _lab(eng.tensor_sub(t1[:, 39:40, :], v[:, 0:1, :], v[:, 37:38, :]), f"t1c{tag}")
                # m = t1 * roll(v,1)
                _lab(eng.tensor_mul(m[:, 0:1, :], t1[:, 0:1, :], v[:, 39:40, :]), f"ma{tag}")
                _lab(eng.tensor_mul(m[:, 1:40, :], t1[:, 1:40, :], v[:, 0:39, :]), f"mb{tag}")

            def shifts_g(st, v, tag=""):
                # row-major fp32: slices along the last (dim) axis.
                t1, m = st["t1"], fresh_m(st, False)
                eng = nc.gpsimd
                _lab(eng.tensor_sub(t1[:, :, 0:2], v[:, :, 1:3], v[:, :, 38:40]), f"t1a{tag}")
                _lab(eng.tensor_sub(t1[:, :, 2:39], v[:, :, 3:40], v[:, :, 0:37]), f"t1b{tag}")
                _lab(eng.tensor_sub(t1[:, :, 39:40], v[:, :, 0:1], v[:, :, 37:38]), f"t1c{tag}")
                _lab(eng.tensor_mul(m[:, :, 0:1], t1[:, :, 0:1], v[:, :, 39:40]), f"ma{tag}")
                _lab(eng.tensor_mul(m[:, :, 1:40], t1[:, :, 1:40], v[:, :, 0:39]), f"mb{tag}")

            all_states = [(st, nc.vector) for st in dstates] + \
                         [(st, nc.gpsimd) for st in gstates]
            if act_interleave:
                na, nb = len(dstates), len(gstates)
                order = []
                for k in range(max(na, nb)):
                    if k < na:
                        order.append(all_states[k])
                    if k < nb:
                        order.append(all_states[na + k])
                act_states = order
            else:
                act_states = all_states

            # ---------------- time stepping ------------------------------
            # DVE chunk 0 (PE-assisted): TensorE accumulates its
            # A = sum_i (g_i a_i) m_i in PSUM via scaled-identity matmuls;
            # ACT extracts B = A + sum_i g_i b_i at stage 4 and the step
            # ends with one DVE add (s' = s + B).  Other chunks keep the
            # ACT-z path with the A-accumulation lagging a stage so
            # `A += z` never waits on ACT.  Pool w is self-served on Pool
            # (TensorScalarPtr).  ACT ops are chained in emission order.
            mult = mybir.AluOpType.mult
            add = mybir.AluOpType.add
            chain = _ActChain()

            def is_pe(st):
                return pe_assist and st is dstates[0]

            def interleave(states):
                na, nb = len(dstates), len(gstates)
                out = []
                for k in range(max(na, nb)):
                    if k < na:
                        out.append(states[k])
                    if k < nb:
                        out.append(states[na + k])
                return out

            rr = interleave(all_states) if act_interleave else list(all_states)
            t0 = 0.0
            for n in range(n_steps):
                dt = dts[n]
                cc = (dt / 2, dt / 2, dt)
                gg = (dt / 6, dt / 3, dt / 3, dt / 6)
                dorder = list(enumerate(dstates))
                for i in range(4):
                    ts = t0 + delta[i] * dt
                    a_i = math.exp(-ts)
                    b_i = F_FORCE * math.exp(ts)
                    # part 1: shifts, plain chunk first: the PE chunk's new
                    # s arrives via the ACT extract at the step boundary, so
                    # giving the plain chunk the head slot hides that.
                    for ci, st in dorder:
                        shifts_d(st, st["s"] if i == 0 else st["y"],
                                 f"_n{n}s{i}d{ci}")
                        if is_pe(st):
                            free = st["C"] * DIM
                            if i == 0:
                                # seed PSUM with s (unscaled identity), so
                                # the stage-4 extract yields s' directly
                                sf = st["s"][:, :, :].rearrange(
                                    "p d c -> p (d c)")
                                for k in range((free + 511) // 512):
                                    lo = k * 512
                                    hi = min(lo + 512, free)
                                    _lab(nc.tensor.matmul(
                                        psumA[:, lo:hi], ident[:, :],
                                        sf[:, lo:hi], start=True,
                                        stop=False), f"mmS_n{n}k{k}")
                            chain(_lab(nc.scalar.activation(
                                wtile[:, :], ident[:, :], Copy,
                                bias=0.0, scale=gg[i] * a_i),
                                f"Wscale_n{n}s{i}"))
                            mf = st["m"][:, :, :].rearrange("p d c -> p (d c)")
                            for k in range((free + 511) // 512):
                                lo, hi = k * 512, min((k + 1) * 512, free)
                                _lab(nc.tensor.matmul(
                                    psumA[:, lo:hi], wtile[:, :],
                                    mf[:, lo:hi],
                                    start=False, stop=(i == 3)),
                                    f"mm_n{n}s{i}k{k}")
                    for ci, st in enumerate(gstates):
                        shifts_g(st, st["s"] if i == 0 else st["y"],
                                 f"_n{n}s{i}g{ci}")
                    # pool w self-served on Pool: its y never waits on ACT
                    if i < 3 and pool_w_self:
                        for st in gstates:
                            _lab(nc.gpsimd.tensor_scalar(
                                st["w"][:, :, :], st["m"][:, :, :],
                                cc[i] * a_i, cc[i] * b_i, mult, add),
                                f"wTS_n{n}s{i}_{st['off']}")
                    # ACT w (critical path); the non-PE DVE chunk's w is
                    # split in dim-halves so its y can start earlier
                    if i < 3:
                        w_states = ([st for _, st in dorder]
                                    if pool_w_self else [s for s, _ in rr])
                        for st in w_states:
                            chain(_lab(nc.scalar.activation(
                                st["w"][:, :, :], st["m"][:, :, :],
                                Copy, bias=cc[i] * b_i,
                                scale=cc[i] * a_i),
                                f"w_n{n}s{i}_{st['off']}"))
                    # y updates, then lagged A += z (late z must not block y)
                    if i < 3:
                        y_order = ([(st, nc.vector) for _, st in dorder]
                                   + [(st, nc.gpsimd) for st in gstates])
                        for st, eng in y_order:
                            _lab(eng.tensor_add(
                                st["y"][:, :, :], st["s"][:, :, :],
                                st["w"][:, :, :]), f"y_n{n}s{i}_{st['off']}")
                    if i >= 2:
                        for st, eng in all_states:
                            if is_pe(st):
                                continue
                            _lab(eng.tensor_add(
                                st["A"][:, :, :], st["A"][:, :, :],
                                st["z"][:, :, :]), f"Aadd_n{n}s{i}_{st['off']}")
                    # z affines for the non-PE chunks (consumed a stage
                    # later).  Stage-4 z of the plain DVE chunk runs on DVE
                    # (4x tensor_scalar) so the step-end s' never waits ACT.
                    for st, _ in rr:
                        if is_pe(st):
                            continue
                        if i == 3 and st in dstates:
                            _lab(nc.vector.tensor_scalar(
                                st["z"][:, :, :], st["m"][:, :, :],
                                gg[i] * a_i, gg[i] * b_i, mult, add),
                                f"zTS_n{n}s{i}_{st['off']}")
                            continue
                        zdst = st["A"] if i == 0 else st["z"]
                        chain(_lab(nc.scalar.activation(
                            zdst[:, :, :], st["m"][:, :, :], Copy,
                            bias=gg[i] * b_i, scale=gg[i] * a_i),
                            f"z_n{n}s{i}_{st['off']}"))
                    if i == 3 and pe_assist:
                        kbar = sum(
                            gg[j] * F_FORCE * math.exp(t0 + delta[j] * dt)
                            for j in range(4))
                        stp = dstates[0]
                        chain(_lab(nc.scalar.activation(
                            stp["y"][:, :, :].rearrange("p d c -> p (d c)"),
                            psumA[:, :], Copy, bias=kbar, scale=1.0),
                            f"Sex_n{n}"))
                # step end
                t0 += dt
                for st, eng in all_states:
                    if is_pe(st):
                        # s' was written into y by the PSUM extract
                        st["s"], st["y"] = st["y"], st["s"]
                    else:
                        _lab(eng.tensor_add(
                            st["y"][:, :, :], st["s"][:, :, :],
                            st["A"][:, :, :]), f"B_n{n}_{st['off']}")
                        _lab(eng.tensor_add(
                            st["s"][:, :, :], st["y"][:, :, :],
                            st["z"][:, :, :]), f"sfin_n{n}_{st['off']}")

            # ---------------- store ----------------------------------
            # Final state tiles go out as fp16 in their native layouts;
            # the host applies the e^-T unscale during unmarshalling.
            for j, st in enumerate(dstates):
                q = nc.sync if j == 0 else nc.scalar
                q.dma_start(xio[f"yd{j}"][:, :, :], st["s"][:, :, :])
            for j, st in enumerate(gstates):
                nc.scalar.dma_start(xio[f"yg{j}"][:, :, :], st["s"][:, :, :])

    nc.compile()
    return nc


def run(x: np.ndarray, trace: bool = False):
    """Run on the 8 cores; returns (output, BassKernelResults).

    The host marshals inputs/outputs: fp32 [BATCH, DIM] rows are split
    into per-chunk fp16 arrays laid out exactly like the device tiles
    (dim-major [P, DIM, C] for DVE chunks, row-major [P, C, DIM] for the
    Pool chunk), and the e^-T unscale of the integrating factor is applied
    on the way out.
    """
    import os

    from concourse.bass_utils import run_bass_kernel_spmd

    try:
        import antenv.axon_hooks  # noqa: F401
    except ImportError:
        # No NTFF hook in this image: tracing would crash on import, so
        # make sure an inherited BASS_TRACE can't switch it on.
        os.environ.setdefault("BASS_NEVER_TRACE", "1")
        trace = False

    if "nc" not in _CACHE:
        _CACHE["nc"] = build()
    nc = _CACHE["nc"]

    x = np.ascontiguousarray(np.asarray(x, dtype=np.float32))
    assert x.shape == (BATCH, DIM)
    shards = x.reshape(N_CORES, P, RB, DIM)

    chunks = []  # (name, offset, C, dim_major)
    off = 0
    for j, C in enumerate(DVE_CHUNKS):
        chunks.append((f"d{j}", off, C, True))
        off += C
    for j, C in enumerate(GP_CHUNKS):
        chunks.append((f"g{j}", off, C, False))
        off += C
    assert off == RB

    in_maps = []
    for i in range(N_CORES):
        m = {}
        for name, o, C, dim_major in chunks:
            part = shards[i, :, o:o + C, :].astype(np.float16)
            if dim_major:
                part = np.ascontiguousarray(part.transpose(0, 2, 1))
            m[f"x{name}"] = np.ascontiguousarray(part)
        in_maps.append(m)

    res = run_bass_kernel_spmd(nc, in_maps, list(range(N_CORES)), trace=trace)

    out = np.empty((N_CORES, P, RB, DIM), dtype=np.float32)
    scale = np.float32(math.exp(-T_END))
    for i in range(N_CORES):
        r = res.results[i]
        for name, o, C, dim_major in chunks:
            part = r[f"y{name}"].astype(np.float32)
            if dim_major:
                part = part.transpose(0, 2, 1)
            out[i, :, o:o + C, :] = part * scale
    return out.reshape(BATCH, DIM), res


def kernel(x: np.ndarray) -> np.ndarray:
    return run(x)[0]
